# revision 25
# baseline (speedup 1.0000x reference)
"""Trainium2 Bass kernel for nn_Attention_22454089023887 (sparse_attention).

LayerNorm -> QKV -> 8-head attention with gathered rel-pos bias -> softmax -> proj.
Sharding: data-parallel over batch B=32 across 8 cores (4 batches/core), no
collectives.  The bias gather attn_biases[:, bias_idxs] has no efficient device
primitive (GPSIMD gather is ~100x too slow for 8M elements), so exp(bias) is
precomputed host-side and streamed as a bf16 input; the softmax applies it
multiplicatively: exp(s + b) = exp(s) * exp(b).

Device-side layout choices (see comments inline):
  - scores are computed transposed, ST[m, n], so the AV matmul can contract
    over m on partitions with no attention-matrix transpose;
  - softmax Z comes free from a ones-column appended to V (row 64 of AV psum);
  - all LN/QKV bias terms that are softmax-row-constants are dropped; the
    m-varying term scale*bq.k0[m] enters as an extra QKV output channel;
  - v-bias and beta/gamma fold into host-side weight preprocessing.
"""

import sys
import os

sys.path.insert(0, "/opt/trn_rl_repo")

import numpy as np
import ml_dtypes

B, N, DIM = 32, 1024, 256
H, KD, AR = 8, 16, 4
D = AR * KD  # 64
DH = D * H  # 512
SCALE = KD ** (-0.5)  # 0.25
EPS = 1e-5
NCORES = 8
BL = B // NCORES  # 4 batches per core
T = BL * N  # 4096 tokens per core
BH = BL * H  # 32 (b, h) pairs per core

_BF16 = ml_dtypes.bfloat16

_CACHE = {}


def _build():
    """Build the Bass graph once; returns (nc, names) for run_bass_kernel_spmd."""
    import concourse.bass as bass
    import concourse.tile as tile
    from concourse import bacc, mybir
    from concourse.masks import make_identity

    f32 = mybir.dt.float32
    bf16 = mybir.dt.bfloat16
    AF = mybir.ActivationFunctionType
    ALU = mybir.AluOpType

    nc = bacc.Bacc("TRN2", target_bir_lowering=False, debug=False,
                   num_devices=NCORES)

    # ---- DRAM parameters (per-core shards / replicated tables) ----
    x_d = nc.dram_tensor("x", [T, DIM], f32, kind="ExternalInput")
    # lhsT for Q/K-ext matmul: ch-halves folded [128, 2(kk), 512(M)]
    wA_d = nc.dram_tensor("wA", [128, 2, 512], bf16, kind="ExternalInput")
    # rhs for V matmul: [128, 2(kk), 512(N)] col h*64+j = Wv rows
    wV_d = nc.dram_tensor("wV", [128, 2, DH], bf16, kind="ExternalInput")
    # proj lhsT, folded: [128, 4, 256]; contraction halves (see host prep)
    wP_d = nc.dram_tensor("wP", [128, 4, DIM], bf16, kind="ExternalInput")
    bP_d = nc.dram_tensor("bP", [128, 2], f32, kind="ExternalInput")
    ones_d = nc.dram_tensor("ones8", [H, T], bf16, kind="ExternalInput")
    # exp(bias) transposed: [h, m, n] viewed [H*8, 128, 1024]
    eb_d = nc.dram_tensor("ebT", [H * 8, 128, N], bf16, kind="ExternalInput")
    out_d = nc.dram_tensor("outT", [2, 128, T], f32, kind="ExternalOutput")
    ap_d = nc.dram_tensor("attn_pT_dram", [128, 4, T], mybir.dt.bfloat16,
                          kind="Internal")
    rz_d = nc.dram_tensor("rz_dram", [BH, N], mybir.dt.bfloat16,
                          kind="Internal")

    NT = T // 128  # 32 token tiles

    with tile.TileContext(nc) as tc:
        with tc.tile_pool(name="const", bufs=1) as const_pool:
            ident = const_pool.tile([128, 128], bf16)
            make_identity(nc, ident)
            eps_t = const_pool.tile([128, 1], f32)
            nc.vector.memset(eps_t, EPS)
            wP_sb = const_pool.tile([128, 4, DIM], bf16)
            nc.sync.dma_start(out=wP_sb, in_=wP_d.ap())
            bP_sb = const_pool.tile([128, 2], f32)
            nc.sync.dma_start(out=bP_sb, in_=bP_d.ap())

            # Persistent activations
            qks = [const_pool.tile([128, 4, 512], bf16, name=f"qks{_j}")
                   for _j in range(8)]   # [Qext;Kext] rows, j-chunks
            vts = [const_pool.tile([128, 8, 65], bf16, name=f"vts{_t}")
                   for _t in range(NT)]  # v per tok-tile (+ones col)

            # ---------- Phases A-C (xnT and matmul weights are scoped) -----
            acts_ctx = tc.tile_pool(name="acts", bufs=1)
            acts_pool = acts_ctx.__enter__()
            wA_sb = acts_pool.tile([128, 2, 512], bf16, tag="wA")
            nc.sync.dma_start(out=wA_sb, in_=wA_d.ap())
            wV_sb = acts_pool.tile([128, 2, DH], bf16, tag="wV")
            nc.sync.dma_start(out=wV_sb, in_=wV_d.ap())
            xnTs = [acts_pool.tile([128, 2, 512], bf16, tag=f"xnT{_j}",
                                   name=f"xnT{_j}") for _j in range(8)]
            # ---------- Phase A: LayerNorm (token layout) + PE transpose ----
            with tc.tile_pool(name="ln", bufs=3) as ln_pool, \
                 tc.tile_pool(name="lnp", bufs=4, space="PSUM") as lnp_pool:
                for t in range(NT):
                    x_t = ln_pool.tile([128, DIM], f32, tag="x")
                    nc.sync.dma_start(out=x_t, in_=x_d.ap()[t * 128:(t + 1) * 128, :])
                    stats = ln_pool.tile([128, 6], f32, tag="st")
                    nc.vector.bn_stats(out=stats, in_=x_t)
                    mv = ln_pool.tile([128, 2], f32, tag="mv")
                    nc.vector.bn_aggr(out=mv, in_=stats)
                    # rstd = 1/sqrt(var+eps)
                    std = ln_pool.tile([128, 1], f32, tag="sd")
                    nc.scalar.activation(out=std, in_=mv[:, 1:2], func=AF.Sqrt,
                                         bias=eps_t, scale=1.0)
                    nc.vector.reciprocal(out=std, in_=std)
                    xn_t = ln_pool.tile([128, DIM], bf16, tag="xn")
                    nc.vector.tensor_scalar(out=xn_t, in0=x_t,
                                            scalar1=mv[:, 0:1], scalar2=std,
                                            op0=ALU.subtract, op1=ALU.mult)
                    for kk in range(2):
                        ps_t = lnp_pool.tile([128, 128], bf16, tag="tp")
                        nc.tensor.transpose(ps_t, xn_t[:, kk * 128:(kk + 1) * 128],
                                            ident)
                        nc.vector.tensor_copy(
                            out=xnTs[t // 4][:, kk,
                                             (t % 4) * 128:(t % 4 + 1) * 128],
                            in_=ps_t)

            # ---------- Phase B: Q/K-ext matmul  out[512, T] ----------------
            with tc.tile_pool(name="qkp", bufs=4, space="PSUM") as qkp_pool:
                for j in range(T // 512):
                    for m in range(4):
                        ps = qkp_pool.tile([128, 512], f32, tag="qk")
                        for kk in range(2):
                            nc.tensor.matmul(
                                ps,
                                lhsT=wA_sb[:, kk, m * 128:(m + 1) * 128],
                                rhs=xnTs[j][:, kk, :],
                                start=(kk == 0), stop=(kk == 1))
                        nc.scalar.activation(
                            out=qks[j][:, m, :], in_=ps, func=AF.Copy)
                    # ones rows for the q-side bias-correction channel
                    for h in range(H):
                        p = (h * 32 + 16) % 128
                        blk = (h * 32) // 128
                        nc.sync.dma_start(
                            out=qks[j][p:p + 1, blk, :],
                            in_=ones_d.ap()[h:h + 1, j * 512:(j + 1) * 512])

            # ---------- Phase C: V matmul  v_tok[T, 512] (+ones cols) -------
            with tc.tile_pool(name="vp", bufs=4, space="PSUM") as vp_pool:
                for t in range(NT):
                    ps = vp_pool.tile([128, 512], f32, tag="v")
                    for kk in range(2):
                        nc.tensor.matmul(
                            ps,
                            lhsT=xnTs[t // 4][:, kk,
                                              (t % 4) * 128:(t % 4 + 1) * 128],
                            rhs=wV_sb[:, kk, :],
                            start=(kk == 0), stop=(kk == 1))
                    # spread heads into 65-wide blocks (col 64 = ones)
                    nc.vector.tensor_copy(
                        out=vts[t][:, :, 0:64],
                        in_=ps.rearrange("p (h d) -> p h d", h=8))
                    nc.vector.memset(vts[t][:, :, 64:65], 1.0)

            acts_ctx.__exit__(None, None, None)

            # ---------- Phase D: attention, joint head pairs ----------------
            # Scores at [128,1024] granularity with 3-deep PSUM pipelining;
            # AV runs nc2-sequential so its accumulators only hold 2 banks.
            # Z: ones-column of V -> row 64 of AV psum -> partition-spread
            # reciprocal -> DRAM bounce -> gpsimd replication DMA -> in-place
            # normalize of the drained AV tile -> store to DRAM.
            with tc.tile_pool(name="eb", bufs=6) as eb_pool, \
                 tc.tile_pool(name="at", bufs=3) as at_pool, \
                 tc.tile_pool(name="avs", bufs=5) as avs_pool, \
                 tc.tile_pool(name="zp", bufs=3) as z_pool, \
                 tc.tile_pool(name="rzp", bufs=3) as rz_pool, \
                 tc.tile_pool(name="sp", bufs=2, space="PSUM") as sp_pool, \
                 tc.tile_pool(name="avp", bufs=2, space="PSUM") as avp_pool:
                for hp in range(H // 2):
                    ebh = {}
                    for h2 in range(2):
                        for hf in range(2):
                            t_eb = eb_pool.tile([128, 4, N], bf16, tag="eb",
                                                name=f"eb{hp}_{h2}_{hf}")
                            ebh[(h2, hf)] = t_eb
                            c0 = hp * 16 + h2 * 8 + hf * 4
                            nc.sync.dma_start(
                                out=t_eb,
                                in_=eb_d.ap()[c0:c0 + 4, :, :]
                                .rearrange("c p n -> p c n"))
                    for b in range(BL):
                        # sequential heads; scores chunked into [128,1536]
                        # psum tiles so each ACT exp amortizes its overhead
                        av_sbs = {}
                        zsp = z_pool.tile([128, 16], bf16, tag="zsp")
                        for h2 in range(2):
                            h = hp * 2 + h2
                            base = 32 * (h % 4)
                            blkq = h // 4
                            at_t = at_pool.tile([128, 8, N], bf16, tag="at")
                            # 16 chunks of 512 cols -> 6 psum tiles of 1536
                            pss = None
                            for g in range(16):
                                mt, nc2 = g // 2, g % 2
                                k, off = (g * 512) // 1536, (g * 512) % 1536
                                if off == 0:
                                    pss = sp_pool.tile([128, 1536], f32,
                                                       tag="s", name=f"s{k}")
                                jm = (b * N + mt * 128) // 512
                                om = (b * N + mt * 128) % 512
                                width = min(1536 - off, 512)
                                nc.tensor.matmul(
                                    pss[:, off:off + 512],
                                    lhsT=qks[jm][base:base + 32, 2 + blkq,
                                                 om:om + 128],
                                    rhs=qks[2 * b + nc2][base:base + 32,
                                                         blkq, :],
                                    start=True, stop=True,
                                    tile_position=(base, 0))
                                if off + 512 == 1536 or g == 15:
                                    w = off + 512
                                    nc.scalar.activation(
                                        out=at_t.rearrange("p a n -> p (a n)")
                                        [:, k * 1536:k * 1536 + w],
                                        in_=pss[:, 0:w], func=AF.Exp)
                            for q in range(2):
                                nc.vector.tensor_mul(
                                    out=at_t[:, q * 4:(q + 1) * 4, :]
                                    .rearrange("p a n -> p (a n)"),
                                    in0=at_t[:, q * 4:(q + 1) * 4, :]
                                    .rearrange("p a n -> p (a n)"),
                                    in1=ebh[(h2, q)]
                                    .rearrange("p a n -> p (a n)"))
                            for nc2 in range(2):
                                ps_av = avp_pool.tile([65, 512], f32, tag="av")
                                for mt in range(8):
                                    # chunk g=(mt,nc2) lives at at-cols
                                    # mt*1024 + nc2*512 (coincides with the
                                    # natural (mt, n) layout)
                                    nc.tensor.matmul(
                                        ps_av,
                                        lhsT=vts[b * 8 + mt][:, h, :],
                                        rhs=at_t[:, mt, nc2 * 512:
                                                 (nc2 + 1) * 512],
                                        start=(mt == 0), stop=(mt == 7))
                                av_sb = avs_pool.tile([65, 512], bf16,
                                                      tag="avs")
                                av_sbs[(h2, nc2)] = av_sb
                                nc.any.tensor_copy(out=av_sb, in_=ps_av)
                                nc.sync.dma_start(
                                    out=zsp[nc2 * 64:(nc2 + 1) * 64,
                                            h2 * 8:(h2 + 1) * 8],
                                    in_=av_sb[64:65, :])
                        with nc.allow_low_precision(reason="1/Z bf16 ok"):
                            nc.vector.reciprocal(out=zsp, in_=zsp)
                        for h2 in range(2):
                            h = hp * 2 + h2
                            nc.sync.dma_start(
                                out=rz_d.ap()[b * H + h:b * H + h + 1, :],
                                in_=zsp[:, h2 * 8:(h2 + 1) * 8])
                        for h2 in range(2):
                            h = hp * 2 + h2
                            rz = rz_pool.tile([64, N], bf16, tag="rz")
                            src = rz_d.ap()[b * H + h:b * H + h + 1, :]
                            nc.gpsimd.dma_start(
                                out=rz,
                                in_=bass.AP(tensor=src.tensor, offset=src.offset,
                                            ap=[[0, 64]] + src.ap[1:]))
                            for nc2 in range(2):
                                nc.vector.tensor_mul(
                                    out=av_sbs[(h2, nc2)][0:64, :],
                                    in0=av_sbs[(h2, nc2)][0:64, :],
                                    in1=rz[:, nc2 * 512:(nc2 + 1) * 512])
                                nc.sync.dma_start(
                                    out=ap_d.ap()[(h % 2) * 64:(h % 2) * 64 + 64,
                                                  h // 2,
                                                  b * N + nc2 * 512:
                                                  b * N + (nc2 + 1) * 512],
                                    in_=av_sbs[(h2, nc2)][0:64, :])

            # ---------- Phase E: output projection --------------------------
            with tc.tile_pool(name="po", bufs=3) as po_pool, \
                 tc.tile_pool(name="pp", bufs=4, space="PSUM") as pp_pool:
                for j in range(T // 512):
                    apt = po_pool.tile([128, 4, 512], bf16, tag="apt")
                    nc.sync.dma_start(out=apt,
                                      in_=ap_d.ap()[:, :, j * 512:(j + 1) * 512])
                    for mo in range(2):
                        ps = pp_pool.tile([128, 512], f32, tag="p")
                        for kk in range(4):
                            nc.tensor.matmul(
                                ps,
                                lhsT=wP_sb[:, kk, mo * 128:(mo + 1) * 128],
                                rhs=apt[:, kk, :],
                                start=(kk == 0), stop=(kk == 3))
                        o_sb = po_pool.tile([128, 512], f32, tag="o")
                        nc.vector.tensor_scalar(
                            out=o_sb, in0=ps,
                            scalar1=bP_sb[:, mo:mo + 1],
                            scalar2=None, op0=ALU.add)
                        nc.sync.dma_start(
                            out=out_d.ap()[mo, :, j * 512:(j + 1) * 512],
                            in_=o_sb)

    nc.compile()
    return nc


def _host_prep(gamma, beta, w_qkv, b_qkv, w_proj, b_proj, attn_biases,
               bias_idxs):
    """Fold biases/affines into weights; gather+exp the bias table."""
    w_eff = (w_qkv * gamma[None, :]).astype(np.float32)
    b_eff = (w_qkv @ beta + b_qkv).astype(np.float32)
    wq = np.zeros((H, KD, DIM), np.float32)
    wk = np.zeros((H, KD, DIM), np.float32)
    wv = np.zeros((H, D, DIM), np.float32)
    bq = np.zeros((H, KD), np.float32)
    bv = np.zeros((H, D), np.float32)
    for h in range(H):
        r0 = h * (2 * KD + D)
        wq[h] = w_eff[r0:r0 + KD]
        wk[h] = w_eff[r0 + KD:r0 + 2 * KD]
        wv[h] = w_eff[r0 + 2 * KD:r0 + 2 * KD + D]
        bq[h] = b_eff[r0:r0 + KD]
        bv[h] = b_eff[r0 + 2 * KD:r0 + 2 * KD + D]

    # wA: [256, 512] cols = Qext | Kext blocks of 32 per head
    wA = np.zeros((DIM, 512), np.float32)
    for h in range(H):
        wA[:, h * 32:h * 32 + KD] = (SCALE * wq[h]).T
        wA[:, 256 + h * 32:256 + h * 32 + KD] = wk[h].T
        # extra channel: scale * (bq_h @ Wk_h)
        wA[:, 256 + h * 32 + KD] = SCALE * (bq[h] @ wk[h])
    wV = np.zeros((DIM, DH), np.float32)
    for h in range(H):
        wV[:, h * D:(h + 1) * D] = wv[h].T
    wA = np.ascontiguousarray(wA.reshape(2, 128, 512).transpose(1, 0, 2))
    wV = np.ascontiguousarray(wV.reshape(2, 128, DH).transpose(1, 0, 2))

    # proj lhsT with the attn_pT folded layout: contraction row (h, d) lives at
    # partition d, free-block h  ->  wP[d, h, c] = w_proj[c, h*64+d]
    wP = np.zeros((128, 4, DIM), np.float32)
    wpr = w_proj.reshape(DIM, H, D)  # [c, h, d]
    for h in range(H):
        wP[(h % 2) * 64:(h % 2) * 64 + 64, h // 2, :] = wpr[:, h, :].T
    bP = np.ascontiguousarray((b_proj + w_proj @ bv.reshape(DH)).astype(np.float32).reshape(2, 128).T)

    ebT = np.exp(attn_biases.astype(np.float32))[:, bias_idxs.T]  # [H, m, n]
    ebT = np.ascontiguousarray(ebT.reshape(H * 8, 128, N)).astype(_BF16)
    ones8 = np.ones((H, T), _BF16)
    return (wA.astype(_BF16), wV.astype(_BF16), wP.astype(_BF16),
            bP.astype(np.float32), ebT, ones8)


def _register_ntff_hook():
    """The container's antenv stub lacks axon_hooks; synthesize it so
    run_bass_kernel_spmd(trace=True) can capture NTFF profiles."""
    import types
    if "antenv.axon_hooks" in sys.modules:
        return
    try:
        from trn_agent_boot.trn_boot import _ntff_profile_via_ctypes
        mod = types.ModuleType("antenv.axon_hooks")
        _state = {"hook": None}
        mod.set_axon_ntff_profile_hook = lambda h: _state.__setitem__("hook", h)
        mod.get_axon_ntff_profile_hook = lambda: _state["hook"]
        sys.modules["antenv.axon_hooks"] = mod
        mod.set_axon_ntff_profile_hook(
            _ntff_profile_via_ctypes("/opt/axon/libaxon_pjrt.so"))
    except Exception:
        pass


def kernel(x, gamma, beta, w_qkv, b_qkv, w_proj, b_proj, attn_biases,
           bias_idxs):
    from concourse.bass_utils import run_bass_kernel_spmd

    x = np.asarray(x, np.float32)
    gamma = np.asarray(gamma, np.float32)
    beta = np.asarray(beta, np.float32)
    w_qkv = np.asarray(w_qkv, np.float32)
    b_qkv = np.asarray(b_qkv, np.float32)
    w_proj = np.asarray(w_proj, np.float32)
    b_proj = np.asarray(b_proj, np.float32)
    attn_biases = np.asarray(attn_biases, np.float32)
    bias_idxs = np.asarray(bias_idxs, np.int32)

    wA, wV, wP, bP, ebT, ones8 = _host_prep(
        gamma, beta, w_qkv, b_qkv, w_proj, b_proj, attn_biases, bias_idxs)

    if "nc" not in _CACHE:
        _CACHE["nc"] = _build()
    nc = _CACHE["nc"]

    in_maps = []
    for c in range(NCORES):
        xs = np.ascontiguousarray(
            x[c * BL:(c + 1) * BL].reshape(T, DIM)).astype(np.float32)
        in_maps.append({
            "x": xs, "wA": wA, "wV": wV, "wP": wP, "bP": bP,
            "ones8": ones8, "ebT": ebT,
        })

    trace = bool(int(os.environ.get("BASS_TRACE_RUN", "0")))
    if trace:
        _register_ntff_hook()
    try:
        res = run_bass_kernel_spmd(nc, in_maps,
                                   core_ids=list(range(NCORES)), trace=trace)
    except Exception:
        if not trace:
            raise
        res = run_bass_kernel_spmd(nc, in_maps,
                                   core_ids=list(range(NCORES)), trace=False)
    _CACHE["last_result"] = res
    outs = []
    for c in range(NCORES):
        oT = res.results[c]["outT"]  # [2, 128, T] f32
        o = oT.reshape(DIM, T).T.reshape(BL, N, DIM)
        outs.append(o)
    return np.concatenate(outs, 0).astype(np.float32)


# revision 28
# speedup vs baseline: 1.1607x; 1.1607x over previous
"""Trainium2 Bass kernel for nn_Attention_22454089023887 (sparse_attention).

LayerNorm -> QKV -> 8-head attention with gathered rel-pos bias -> softmax -> proj.
Sharding: data-parallel over batch B=32 across 8 cores (4 batches/core), no
collectives.  The bias gather attn_biases[:, bias_idxs] has no efficient device
primitive (GPSIMD gather is ~100x too slow for 8M elements), so exp(bias) is
precomputed host-side and streamed as a bf16 input; the softmax applies it
multiplicatively: exp(s + b) = exp(s) * exp(b).

Device-side layout choices (see comments inline):
  - scores are computed transposed, ST[m, n], so the AV matmul can contract
    over m on partitions with no attention-matrix transpose;
  - softmax Z comes free from a ones-column appended to V (row 64 of AV psum);
  - all LN/QKV bias terms that are softmax-row-constants are dropped; the
    m-varying term scale*bq.k0[m] enters as an extra QKV output channel;
  - v-bias and beta/gamma fold into host-side weight preprocessing.
"""

import sys
import os

sys.path.insert(0, "/opt/trn_rl_repo")

import numpy as np
import ml_dtypes

B, N, DIM = 32, 1024, 256
H, KD, AR = 8, 16, 4
D = AR * KD  # 64
DH = D * H  # 512
SCALE = KD ** (-0.5)  # 0.25
EPS = 1e-5
NCORES = 8
BL = B // NCORES  # 4 batches per core
T = BL * N  # 4096 tokens per core
BH = BL * H  # 32 (b, h) pairs per core

_BF16 = ml_dtypes.bfloat16

_CACHE = {}


def _build():
    """Build the Bass graph once; returns (nc, names) for run_bass_kernel_spmd."""
    import concourse.bass as bass
    import concourse.tile as tile
    from concourse import bacc, mybir
    from concourse.masks import make_identity

    f32 = mybir.dt.float32
    bf16 = mybir.dt.bfloat16
    AF = mybir.ActivationFunctionType
    ALU = mybir.AluOpType

    nc = bacc.Bacc("TRN2", target_bir_lowering=False, debug=False,
                   num_devices=NCORES)

    # ---- DRAM parameters (per-core shards / replicated tables) ----
    x_d = nc.dram_tensor("x", [T, DIM], f32, kind="ExternalInput")
    # lhsT for Q/K-ext matmul: ch-halves folded [128, 2(kk), 512(M)]
    wA_d = nc.dram_tensor("wA", [128, 2, 512], bf16, kind="ExternalInput")
    # rhs for V matmul: [128, 2(kk), 512(N)] col h*64+j = Wv rows
    wV_d = nc.dram_tensor("wV", [128, 2, DH], bf16, kind="ExternalInput")
    # proj lhsT, folded: [128, 4, 256]; contraction halves (see host prep)
    wP_d = nc.dram_tensor("wP", [128, 4, DIM], bf16, kind="ExternalInput")
    bP_d = nc.dram_tensor("bP", [128, 2], f32, kind="ExternalInput")
    ones_d = nc.dram_tensor("ones8", [H, T], bf16, kind="ExternalInput")
    # exp(bias) transposed: [h, m, n] viewed [H*8, 128, 1024]
    eb_d = nc.dram_tensor("ebT", [H * 8, 128, N], bf16, kind="ExternalInput")
    out_d = nc.dram_tensor("outT", [2, 128, T], f32, kind="ExternalOutput")
    ap_d = nc.dram_tensor("attn_pT_dram", [128, 4, T], mybir.dt.bfloat16,
                          kind="Internal")
    rz_d = nc.dram_tensor("rz_dram", [BH, N], mybir.dt.bfloat16,
                          kind="Internal")

    NT = T // 128  # 32 token tiles

    with tile.TileContext(nc) as tc:
        with tc.tile_pool(name="const", bufs=1) as const_pool:
            ident = const_pool.tile([128, 128], bf16)
            make_identity(nc, ident)
            eps_t = const_pool.tile([128, 1], f32)
            nc.vector.memset(eps_t, EPS)
            wP_sb = const_pool.tile([128, 4, DIM], bf16)
            nc.sync.dma_start(out=wP_sb, in_=wP_d.ap())
            bP_sb = const_pool.tile([128, 2], f32)
            nc.sync.dma_start(out=bP_sb, in_=bP_d.ap())

            # Persistent activations
            qks = [const_pool.tile([128, 4, 512], bf16, name=f"qks{_j}")
                   for _j in range(8)]   # [Qext;Kext] rows, j-chunks
            vts = [const_pool.tile([128, 8, 65], bf16, name=f"vts{_t}")
                   for _t in range(NT)]  # v per tok-tile (+ones col)

            # ---------- Phases A-C (xnT and matmul weights are scoped) -----
            acts_ctx = tc.tile_pool(name="acts", bufs=1)
            acts_pool = acts_ctx.__enter__()
            wA_sb = acts_pool.tile([128, 2, 512], bf16, tag="wA")
            nc.sync.dma_start(out=wA_sb, in_=wA_d.ap())
            wV_sb = acts_pool.tile([128, 2, DH], bf16, tag="wV")
            nc.sync.dma_start(out=wV_sb, in_=wV_d.ap())
            xnTs = [acts_pool.tile([128, 2, 512], bf16, tag=f"xnT{_j}",
                                   name=f"xnT{_j}") for _j in range(8)]
            # ---------- Phases A+B+C fused per j-chunk ----------------------
            with tc.tile_pool(name="ln", bufs=3) as ln_pool, \
                 tc.tile_pool(name="lnp", bufs=4, space="PSUM") as lnp_pool, \
                 tc.tile_pool(name="qkp", bufs=4, space="PSUM") as qkp_pool:
                for j in range(8):
                    for t in range(4 * j, 4 * j + 4):
                        x_t = ln_pool.tile([128, DIM], f32, tag="x")
                        nc.sync.dma_start(out=x_t,
                                          in_=x_d.ap()[t * 128:(t + 1) * 128, :])
                        stats = ln_pool.tile([128, 6], f32, tag="st")
                        nc.vector.bn_stats(out=stats, in_=x_t)
                        mv = ln_pool.tile([128, 2], f32, tag="mv")
                        nc.vector.bn_aggr(out=mv, in_=stats)
                        std = ln_pool.tile([128, 1], f32, tag="sd")
                        nc.scalar.activation(out=std, in_=mv[:, 1:2],
                                             func=AF.Sqrt, bias=eps_t, scale=1.0)
                        nc.vector.reciprocal(out=std, in_=std)
                        xn_t = ln_pool.tile([128, DIM], bf16, tag="xn")
                        nc.vector.tensor_scalar(out=xn_t, in0=x_t,
                                                scalar1=mv[:, 0:1], scalar2=std,
                                                op0=ALU.subtract, op1=ALU.mult)
                        for kk in range(2):
                            ps_t = lnp_pool.tile([128, 128], bf16, tag="tp")
                            nc.tensor.transpose(
                                ps_t, xn_t[:, kk * 128:(kk + 1) * 128], ident)
                            nc.vector.tensor_copy(
                                out=xnTs[t // 4][:, kk,
                                                 (t % 4) * 128:(t % 4 + 1) * 128],
                                in_=ps_t)
                    # Q/K-ext matmul for this j-chunk
                    for m in range(4):
                        ps = qkp_pool.tile([128, 512], f32, tag="qk")
                        for kk in range(2):
                            nc.tensor.matmul(
                                ps,
                                lhsT=wA_sb[:, kk, m * 128:(m + 1) * 128],
                                rhs=xnTs[j][:, kk, :],
                                start=(kk == 0), stop=(kk == 1))
                        nc.scalar.activation(
                            out=qks[j][:, m, :], in_=ps, func=AF.Copy)
                    for h in range(H):
                        p = (h * 32 + 16) % 128
                        blk = (h * 32) // 128
                        nc.sync.dma_start(
                            out=qks[j][p:p + 1, blk, :],
                            in_=ones_d.ap()[h:h + 1, j * 512:(j + 1) * 512])
                    # V matmul for this j-chunk's 4 token tiles
                    for t in range(4 * j, 4 * j + 4):
                        ps = qkp_pool.tile([128, 512], f32, tag="qk")
                        for kk in range(2):
                            nc.tensor.matmul(
                                ps,
                                lhsT=xnTs[t // 4][:, kk,
                                                  (t % 4) * 128:(t % 4 + 1) * 128],
                                rhs=wV_sb[:, kk, :],
                                start=(kk == 0), stop=(kk == 1))
                        nc.vector.tensor_copy(
                            out=vts[t][:, :, 0:64],
                            in_=ps.rearrange("p (h d) -> p h d", h=8))
                        nc.vector.memset(vts[t][:, :, 64:65], 1.0)

            acts_ctx.__exit__(None, None, None)

            # ---------- Phase D: attention, software-pipelined heads --------
            # scores+exp+ebias of head i+1 are emitted BEFORE the AV matmuls
            # of head i, so PE's in-order stream never blocks the ACT exps.
            with tc.tile_pool(name="eb", bufs=6) as eb_pool, \
                 tc.tile_pool(name="at", bufs=3) as at_pool, \
                 tc.tile_pool(name="avs", bufs=5) as avs_pool, \
                 tc.tile_pool(name="zp", bufs=3) as z_pool, \
                 tc.tile_pool(name="rzp", bufs=3) as rz_pool, \
                 tc.tile_pool(name="sp", bufs=2, space="PSUM") as sp_pool, \
                 tc.tile_pool(name="avp", bufs=2, space="PSUM") as avp_pool:
                ebh = {}
                pair_state = {}

                def load_eb(hp):
                    for h2 in range(2):
                        for hf in range(2):
                            t_eb = eb_pool.tile([128, 4, N], bf16, tag="eb",
                                                name=f"eb{hp}_{h2}_{hf}")
                            ebh[(hp, h2, hf)] = t_eb
                            c0 = hp * 16 + h2 * 8 + hf * 4
                            nc.sync.dma_start(
                                out=t_eb,
                                in_=eb_d.ap()[c0:c0 + 4, :, :]
                                .rearrange("c p n -> p c n"))

                def emit_scores(hp, b, h2):
                    h = hp * 2 + h2
                    base = 32 * (h % 4)
                    blkq = h // 4
                    at_t = at_pool.tile([128, 8, N], bf16, tag="at",
                                        name=f"at{hp}_{b}_{h2}")
                    pss = None
                    for g in range(16):
                        mt, nc2 = g // 2, g % 2
                        k, off = (g * 512) // 1536, (g * 512) % 1536
                        if off == 0:
                            pss = sp_pool.tile([128, 1536], f32, tag="s",
                                               name=f"s{k}")
                        jm = (b * N + mt * 128) // 512
                        om = (b * N + mt * 128) % 512
                        nc.tensor.matmul(
                            pss[:, off:off + 512],
                            lhsT=qks[jm][base:base + 32, 2 + blkq, om:om + 128],
                            rhs=qks[2 * b + nc2][base:base + 32, blkq, :],
                            start=True, stop=True, tile_position=(base, 0))
                        if off + 512 == 1536 or g == 15:
                            w = off + 512
                            nc.scalar.activation(
                                out=at_t.rearrange("p a n -> p (a n)")
                                [:, k * 1536:k * 1536 + w],
                                in_=pss[:, 0:w], func=AF.Exp)
                    for q in range(2):
                        nc.vector.tensor_mul(
                            out=at_t[:, q * 4:(q + 1) * 4, :]
                            .rearrange("p a n -> p (a n)"),
                            in0=at_t[:, q * 4:(q + 1) * 4, :]
                            .rearrange("p a n -> p (a n)"),
                            in1=ebh[(hp, h2, q)].rearrange("p a n -> p (a n)"))
                    return at_t

                def emit_av(hp, b, h2, at_t):
                    h = hp * 2 + h2
                    st = pair_state.setdefault(
                        (hp, b), {"zsp": z_pool.tile([128, 16], bf16,
                                                     tag="zsp",
                                                     name=f"zsp{hp}_{b}"),
                                  "av": {}})
                    for nc2 in range(2):
                        ps_av = avp_pool.tile([65, 512], f32, tag="av",
                                              name=f"av{h2}_{nc2}")
                        for mt in range(8):
                            nc.tensor.matmul(
                                ps_av,
                                lhsT=vts[b * 8 + mt][:, h, :],
                                rhs=at_t[:, mt, nc2 * 512:(nc2 + 1) * 512],
                                start=(mt == 0), stop=(mt == 7))
                        av_sb = avs_pool.tile([65, 512], bf16, tag="avs",
                                              name=f"avsb{h2}_{nc2}")
                        st["av"][(h2, nc2)] = av_sb
                        nc.any.tensor_copy(out=av_sb, in_=ps_av)
                        nc.sync.dma_start(
                            out=st["zsp"][nc2 * 64:(nc2 + 1) * 64,
                                          h2 * 8:(h2 + 1) * 8],
                            in_=av_sb[64:65, :])
                    if h2 == 1:
                        finish_pair(hp, b)

                def finish_pair(hp, b):
                    st = pair_state.pop((hp, b))
                    zsp = st["zsp"]
                    with nc.allow_low_precision(reason="1/Z bf16 ok"):
                        nc.vector.reciprocal(out=zsp, in_=zsp)
                    for h2 in range(2):
                        h = hp * 2 + h2
                        nc.sync.dma_start(
                            out=rz_d.ap()[b * H + h:b * H + h + 1, :],
                            in_=zsp[:, h2 * 8:(h2 + 1) * 8])
                    for h2 in range(2):
                        h = hp * 2 + h2
                        rz = rz_pool.tile([64, N], bf16, tag="rz")
                        src = rz_d.ap()[b * H + h:b * H + h + 1, :]
                        nc.gpsimd.dma_start(
                            out=rz,
                            in_=bass.AP(tensor=src.tensor, offset=src.offset,
                                        ap=[[0, 64]] + src.ap[1:]))
                        for nc2 in range(2):
                            nc.vector.tensor_mul(
                                out=st["av"][(h2, nc2)][0:64, :],
                                in0=st["av"][(h2, nc2)][0:64, :],
                                in1=rz[:, nc2 * 512:(nc2 + 1) * 512])
                            nc.sync.dma_start(
                                out=ap_d.ap()[(h % 2) * 64:(h % 2) * 64 + 64,
                                              h // 2,
                                              b * N + nc2 * 512:
                                              b * N + (nc2 + 1) * 512],
                                in_=st["av"][(h2, nc2)][0:64, :])

                heads = [(hp, b, h2) for hp in range(H // 2)
                         for b in range(BL) for h2 in range(2)]
                pending = None
                for (hp, b, h2) in heads:
                    if b == 0 and h2 == 0:
                        load_eb(hp)
                    at_t = emit_scores(hp, b, h2)
                    if pending is not None:
                        emit_av(*pending)
                    pending = (hp, b, h2, at_t)
                emit_av(*pending)

            # ---------- Phase E: output projection --------------------------
            with tc.tile_pool(name="po", bufs=3) as po_pool, \
                 tc.tile_pool(name="pp", bufs=4, space="PSUM") as pp_pool:
                for j in range(T // 512):
                    apt = po_pool.tile([128, 4, 512], bf16, tag="apt")
                    nc.sync.dma_start(out=apt,
                                      in_=ap_d.ap()[:, :, j * 512:(j + 1) * 512])
                    for mo in range(2):
                        ps = pp_pool.tile([128, 512], f32, tag="p")
                        for kk in range(4):
                            nc.tensor.matmul(
                                ps,
                                lhsT=wP_sb[:, kk, mo * 128:(mo + 1) * 128],
                                rhs=apt[:, kk, :],
                                start=(kk == 0), stop=(kk == 3))
                        o_sb = po_pool.tile([128, 512], f32, tag="o")
                        nc.vector.tensor_scalar(
                            out=o_sb, in0=ps,
                            scalar1=bP_sb[:, mo:mo + 1],
                            scalar2=None, op0=ALU.add)
                        nc.sync.dma_start(
                            out=out_d.ap()[mo, :, j * 512:(j + 1) * 512],
                            in_=o_sb)

    nc.compile()
    return nc


def _host_prep(gamma, beta, w_qkv, b_qkv, w_proj, b_proj, attn_biases,
               bias_idxs):
    """Fold biases/affines into weights; gather+exp the bias table."""
    w_eff = (w_qkv * gamma[None, :]).astype(np.float32)
    b_eff = (w_qkv @ beta + b_qkv).astype(np.float32)
    wq = np.zeros((H, KD, DIM), np.float32)
    wk = np.zeros((H, KD, DIM), np.float32)
    wv = np.zeros((H, D, DIM), np.float32)
    bq = np.zeros((H, KD), np.float32)
    bv = np.zeros((H, D), np.float32)
    for h in range(H):
        r0 = h * (2 * KD + D)
        wq[h] = w_eff[r0:r0 + KD]
        wk[h] = w_eff[r0 + KD:r0 + 2 * KD]
        wv[h] = w_eff[r0 + 2 * KD:r0 + 2 * KD + D]
        bq[h] = b_eff[r0:r0 + KD]
        bv[h] = b_eff[r0 + 2 * KD:r0 + 2 * KD + D]

    # wA: [256, 512] cols = Qext | Kext blocks of 32 per head
    wA = np.zeros((DIM, 512), np.float32)
    for h in range(H):
        wA[:, h * 32:h * 32 + KD] = (SCALE * wq[h]).T
        wA[:, 256 + h * 32:256 + h * 32 + KD] = wk[h].T
        # extra channel: scale * (bq_h @ Wk_h)
        wA[:, 256 + h * 32 + KD] = SCALE * (bq[h] @ wk[h])
    wV = np.zeros((DIM, DH), np.float32)
    for h in range(H):
        wV[:, h * D:(h + 1) * D] = wv[h].T
    wA = np.ascontiguousarray(wA.reshape(2, 128, 512).transpose(1, 0, 2))
    wV = np.ascontiguousarray(wV.reshape(2, 128, DH).transpose(1, 0, 2))

    # proj lhsT with the attn_pT folded layout: contraction row (h, d) lives at
    # partition d, free-block h  ->  wP[d, h, c] = w_proj[c, h*64+d]
    wP = np.zeros((128, 4, DIM), np.float32)
    wpr = w_proj.reshape(DIM, H, D)  # [c, h, d]
    for h in range(H):
        wP[(h % 2) * 64:(h % 2) * 64 + 64, h // 2, :] = wpr[:, h, :].T
    bP = np.ascontiguousarray((b_proj + w_proj @ bv.reshape(DH)).astype(np.float32).reshape(2, 128).T)

    ebT = np.exp(attn_biases.astype(np.float32))[:, bias_idxs.T]  # [H, m, n]
    ebT = np.ascontiguousarray(ebT.reshape(H * 8, 128, N)).astype(_BF16)
    ones8 = np.ones((H, T), _BF16)
    return (wA.astype(_BF16), wV.astype(_BF16), wP.astype(_BF16),
            bP.astype(np.float32), ebT, ones8)


def _register_ntff_hook():
    """The container's antenv stub lacks axon_hooks; synthesize it so
    run_bass_kernel_spmd(trace=True) can capture NTFF profiles."""
    import types
    if "antenv.axon_hooks" in sys.modules:
        return
    try:
        from trn_agent_boot.trn_boot import _ntff_profile_via_ctypes
        mod = types.ModuleType("antenv.axon_hooks")
        _state = {"hook": None}
        mod.set_axon_ntff_profile_hook = lambda h: _state.__setitem__("hook", h)
        mod.get_axon_ntff_profile_hook = lambda: _state["hook"]
        sys.modules["antenv.axon_hooks"] = mod
        mod.set_axon_ntff_profile_hook(
            _ntff_profile_via_ctypes("/opt/axon/libaxon_pjrt.so"))
    except Exception:
        pass


def kernel(x, gamma, beta, w_qkv, b_qkv, w_proj, b_proj, attn_biases,
           bias_idxs):
    from concourse.bass_utils import run_bass_kernel_spmd

    x = np.asarray(x, np.float32)
    gamma = np.asarray(gamma, np.float32)
    beta = np.asarray(beta, np.float32)
    w_qkv = np.asarray(w_qkv, np.float32)
    b_qkv = np.asarray(b_qkv, np.float32)
    w_proj = np.asarray(w_proj, np.float32)
    b_proj = np.asarray(b_proj, np.float32)
    attn_biases = np.asarray(attn_biases, np.float32)
    bias_idxs = np.asarray(bias_idxs, np.int32)

    wA, wV, wP, bP, ebT, ones8 = _host_prep(
        gamma, beta, w_qkv, b_qkv, w_proj, b_proj, attn_biases, bias_idxs)

    if "nc" not in _CACHE:
        _CACHE["nc"] = _build()
    nc = _CACHE["nc"]

    in_maps = []
    for c in range(NCORES):
        xs = np.ascontiguousarray(
            x[c * BL:(c + 1) * BL].reshape(T, DIM)).astype(np.float32)
        in_maps.append({
            "x": xs, "wA": wA, "wV": wV, "wP": wP, "bP": bP,
            "ones8": ones8, "ebT": ebT,
        })

    trace = bool(int(os.environ.get("BASS_TRACE_RUN", "0")))
    if trace:
        _register_ntff_hook()
    try:
        res = run_bass_kernel_spmd(nc, in_maps,
                                   core_ids=list(range(NCORES)), trace=trace)
    except Exception:
        if not trace:
            raise
        res = run_bass_kernel_spmd(nc, in_maps,
                                   core_ids=list(range(NCORES)), trace=False)
    _CACHE["last_result"] = res
    outs = []
    for c in range(NCORES):
        oT = res.results[c]["outT"]  # [2, 128, T] f32
        o = oT.reshape(DIM, T).T.reshape(BL, N, DIM)
        outs.append(o)
    return np.concatenate(outs, 0).astype(np.float32)
